# revision 14
# baseline (speedup 1.0000x reference)
"""MinkowskiInstanceNorm on 8 Trainium2 NeuronCores.

Strategy: batch_ids are sorted, so the 16 instances are contiguous row
ranges.  Assign 2 instances per core (no cross-core collectives).  On
host, pad each instance to a uniform length L (zeros) so the SPMD
program is identical across cores.  Per instance on device:
  pass 1: DMA rows into SBUF fp32 panels (each partition holds a
          contiguous row range -> fully contiguous HBM transfers).
          ScalarE casts each panel to a bf16 shadow with a ones column
          per 128-col wide tile; TensorE then computes Gram + sums in a
          single cheap bf16 matmul per wide tile, accumulated in PSUM
          (diag = per-(group,channel) sum of squares, ones col = sums).
          Zero padding contributes nothing.
  stats:  extract diag + sums, fold the 4 row groups per channel with
          two tiny matmuls, compute A = weight/sqrt(var+eps),
          B = bias - mean*A, broadcast to [128, 128] via a K=1 matmul.
  pass 2: y = x*A + B with two VectorE tensor_tensor ops per panel
          (in place on the fp32 panel), then DMA out.  Rows are read
          from HBM once and written once (~64 MB per core).
"""

import numpy as np

import concourse.bass as bass
import concourse.mybir as mybir
import concourse.tile as tile
import concourse.tile_utils as tile_utils
from concourse.alu_op_type import AluOpType
from concourse.bass_utils import run_bass_kernel_spmd

F32 = mybir.dt.float32
BF16 = mybir.dt.bfloat16
P = 128            # SBUF partitions
C = 32             # channels
IPC = 2            # instances per core
NCORES = 8
N_INSTANCES = 16
EPS = 1e-8
ROWS_PP = 16384    # rows per full panel (128 rows per partition, 2 MiB)

# use the full usable SBUF (the default cap leaves 16 KiB/partition unused)
tile_utils.max_sbuf_usage = 208 * 1024

_PROG_CACHE: dict = {}


def _build_program(L: int):
    """Build the SPMD bass program for instance slot length L (mult of 512)."""
    n_panels = (L + ROWS_PP - 1) // ROWS_PP
    nc = bass.Bass()

    x_ext = nc.declare_dram_parameter("x", [IPC, L, C], F32, isOutput=False)
    w_ext = nc.declare_dram_parameter("wvec", [1, C], F32, isOutput=False)
    b_ext = nc.declare_dram_parameter("bvec", [1, C], F32, isOutput=False)
    cinv_ext = nc.declare_dram_parameter("cinv", [1, IPC], F32, isOutput=False)
    g4_ext = nc.declare_dram_parameter("g4", [P, C], F32, isOutput=False)
    dm_ext = nc.declare_dram_parameter("dmask", [P, P], F32, isOutput=False)
    y_ext = nc.declare_dram_parameter("y", [IPC, L, C], F32, isOutput=True)

    with tile.TileContext(nc) as tc:
        with (
            tc.tile_pool(name="panels", bufs=12) as panels,
            tc.tile_pool(name="bfp", bufs=2) as bfp,
            tc.tile_pool(name="consts", bufs=1) as consts,
            tc.tile_pool(name="small", bufs=2) as small,
            tc.tile_pool(name="psum_acc", bufs=2, space="PSUM") as psum_acc,
            tc.tile_pool(name="psum_sm", bufs=1, space="PSUM") as psum_sm,
        ):
            w_sb = consts.tile([1, C], F32, tag="w")
            nc.sync.dma_start(out=w_sb[:], in_=w_ext[:])
            b_sb = consts.tile([1, C], F32, tag="b")
            nc.sync.dma_start(out=b_sb[:], in_=b_ext[:])
            cinv_sb = consts.tile([1, IPC], F32, tag="cinv")
            nc.sync.dma_start(out=cinv_sb[:], in_=cinv_ext[:])
            g4_sb = consts.tile([P, C], F32, tag="g4")
            nc.sync.dma_start(out=g4_sb[:], in_=g4_ext[:])
            dm_sb = consts.tile([P, P], F32, tag="dmask")
            nc.sync.dma_start(out=dm_sb[:], in_=dm_ext[:])
            ones_row = consts.tile([1, P], F32, tag="ones_row")
            nc.vector.memset(ones_row[:], 1.0)
            eps_sb = consts.tile([1, 1], F32, tag="eps")
            nc.vector.memset(eps_sb[:], float(EPS))

            for i in range(IPC):
                # ---- pass 1: load, cast to bf16, accumulate Gram+sums ----
                acc_ps = psum_acc.tile([P, P + 1], F32, tag="acc")
                ptiles = []
                total_wt = L // 512
                wt_done = 0
                for p in range(n_panels):
                    r0 = p * ROWS_PP
                    rows = min(ROWS_PP, L - r0)
                    rpp = rows // P       # rows per partition
                    nwt = rpp // 4        # wide tiles (128 cols each)
                    pt = panels.tile([P, (ROWS_PP // P) * C], F32, tag="panel")
                    src = x_ext[i, r0 : r0 + rows, :].rearrange(
                        "(q n) c -> q (n c)", q=P
                    )
                    nc.gpsimd.dma_start(out=pt[:, : rpp * C], in_=src)

                    ptiles.append((pt, r0, rows, rpp, nwt))

                    # cast to bf16 in half-panel chunks so PE can start
                    # sooner and stream without gaps (stays HAM-warm)
                    CH = 16  # wide tiles per cast chunk
                    pt3 = pt[:].rearrange("q (n k) -> q n k", k=P)
                    for c0 in range(0, nwt, CH):
                        cw = min(CH, nwt - c0)
                        bt = bfp.tile([P, CH * 129], BF16, tag="bf")
                        bt3 = bt[:].rearrange("q (n k) -> q n k", k=129)
                        nc.vector.memset(bt3[:, :cw, 128:129], 1.0)
                        nc.scalar.copy(
                            bt3[:, :cw, 0:P], pt3[:, c0 : c0 + cw, :]
                        )
                        for wt in range(cw):
                            nc.tensor.matmul(
                                acc_ps[:],
                                bt[:, wt * 129 : wt * 129 + P],
                                bt[:, wt * 129 : wt * 129 + P + 1],
                                start=(wt_done == 0),
                                stop=(wt_done == total_wt - 1),
                            )
                            wt_done += 1
                assert wt_done == total_wt

                # ---- stats ----
                ds_sb = small.tile([P, 2], F32, tag="ds")
                scratch = psum_sm.tile([P, P], F32, tag="scratch")
                nc.vector.tensor_tensor(
                    scratch[:], acc_ps[:, 0:P], dm_sb[:], AluOpType.mult
                )
                nc.vector.tensor_reduce(
                    ds_sb[:, 0:1], scratch[:], mybir.AxisListType.X, AluOpType.add
                )
                nc.vector.tensor_copy(ds_sb[:, 1:2], acc_ps[:, P : P + 1])

                d_ps = psum_sm.tile([1, C], F32, tag="d")
                s_ps = psum_sm.tile([1, C], F32, tag="s")
                nc.tensor.matmul(
                    d_ps[:], ds_sb[:, 0:1], g4_sb[:], start=True, stop=True
                )
                nc.tensor.matmul(
                    s_ps[:], ds_sb[:, 1:2], g4_sb[:], start=True, stop=True
                )

                cinv_i = cinv_sb[0:1, i : i + 1]
                mean_sb = small.tile([1, C], F32, tag="mean")
                ex2_sb = small.tile([1, C], F32, tag="ex2")
                var_sb = small.tile([1, C], F32, tag="var")
                std_sb = small.tile([1, C], F32, tag="std")
                istd_sb = small.tile([1, C], F32, tag="istd")
                tmp_sb = small.tile([1, C], F32, tag="tmp")
                ab4_sb = small.tile([1, 8 * C], F32, tag="ab4")

                nc.vector.tensor_scalar_mul(mean_sb[:], s_ps[:], cinv_i)
                nc.vector.tensor_scalar_mul(ex2_sb[:], d_ps[:], cinv_i)
                nc.vector.tensor_mul(var_sb[:], mean_sb[:], mean_sb[:])
                nc.vector.tensor_sub(var_sb[:], ex2_sb[:], var_sb[:])
                nc.scalar.activation(
                    std_sb[:],
                    var_sb[:],
                    mybir.ActivationFunctionType.Sqrt,
                    bias=eps_sb[:],
                )
                nc.vector.reciprocal(istd_sb[:], std_sb[:])
                nc.vector.tensor_mul(ab4_sb[:, 0:C], istd_sb[:], w_sb[:])
                nc.vector.tensor_mul(tmp_sb[:], mean_sb[:], ab4_sb[:, 0:C])
                nc.vector.tensor_sub(ab4_sb[:, 4 * C : 5 * C], b_sb[:], tmp_sb[:])
                # replicate A to slots 1..3 and B to slots 5..7
                nc.vector.tensor_copy(ab4_sb[:, C : 2 * C], ab4_sb[:, 0:C])
                nc.vector.tensor_copy(ab4_sb[:, 2 * C : 4 * C], ab4_sb[:, 0 : 2 * C])
                nc.vector.tensor_copy(
                    ab4_sb[:, 5 * C : 6 * C], ab4_sb[:, 4 * C : 5 * C]
                )
                nc.vector.tensor_copy(
                    ab4_sb[:, 6 * C : 8 * C], ab4_sb[:, 4 * C : 6 * C]
                )

                # broadcast [1, 256] -> [128, 256] via K=1 matmul
                ab_ps = psum_sm.tile([P, 8 * C], F32, tag="abps")
                nc.tensor.matmul(
                    ab_ps[:], ones_row[:], ab4_sb[:], start=True, stop=True
                )
                ab_rep = small.tile([P, 8 * C], F32, tag="abrep")
                nc.scalar.copy(ab_rep[:], ab_ps[:])
                a_wide = ab_rep[:, 0 : 4 * C]        # [128, 128]
                b_wide = ab_rep[:, 4 * C : 8 * C]    # [128, 128]

                # ---- pass 2: normalize in place + store ----
                for pt, r0, rows, rpp, nwt in ptiles:
                    pv = pt[:].rearrange("q (n k) -> q n k", k=P)[:, :nwt, :]
                    a_b = a_wide[:, None, :].broadcast_to([P, nwt, P])
                    b_b = b_wide[:, None, :].broadcast_to([P, nwt, P])
                    nc.vector.tensor_tensor(pv, pv, a_b, AluOpType.mult)
                    nc.vector.tensor_tensor(pv, pv, b_b, AluOpType.add)
                    dst = y_ext[i, r0 : r0 + rows, :].rearrange(
                        "(q n) c -> q (n c)", q=P
                    )
                    nc.sync.dma_start(out=dst, in_=pt[:, : rpp * C])

    # Populate .instr bytes for extended-inst InstISA subclasses — raw bass
    # skips this pass and the NEFF compiler fails with "ISA wrong length"
    # on empty .instr.
    mybir.codegen_inst_isa_subclasses(nc)
    _split_waits(nc)
    return nc


def _split_waits(nc, max_waits: int = 1):
    """This container's walrus rejects instructions carrying more than one
    semaphore wait ("Too many sync wait commands").  Hoist extra waits onto
    same-engine InstNoOps inserted just before the instruction.
    """
    for f in nc.m.functions:
        for blk in f.blocks:
            new = []
            for inst in blk.instructions:
                si = inst.sync_info
                if (
                    si is not None
                    and len(si.on_wait) > max_waits
                    and not isinstance(inst, mybir.InstNoOp)
                ):
                    waits = list(si.on_wait)
                    for w in waits[:-max_waits]:
                        nop = mybir.InstNoOp(
                            name=nc.get_next_instruction_name(),
                            engine=inst.engine,
                            sync_info=mybir.SyncInfo(on_wait=[w], on_update=[]),
                            bass_nofuse=True,
                        )
                        new.append(nop)
                    inst.sync_info = mybir.SyncInfo(
                        on_wait=waits[-max_waits:], on_update=list(si.on_update)
                    )
                new.append(inst)
            blk.instructions[:] = new


def _get_program(L: int):
    prog = _PROG_CACHE.get(L)
    if prog is None:
        prog = _build_program(L)
        _PROG_CACHE[L] = prog
    return prog


def _run(feat, batch_ids, weight, bias, trace=False, trace_kwargs=None):
    feat = np.ascontiguousarray(np.asarray(feat, dtype=np.float32))
    batch_ids = np.asarray(batch_ids, dtype=np.int32)
    weight = np.asarray(weight, dtype=np.float32).reshape(1, C)
    bias = np.asarray(bias, dtype=np.float32).reshape(1, C)
    n = feat.shape[0]

    perm = None
    if np.any(np.diff(batch_ids) < 0):  # insurance; spec says sorted
        perm = np.argsort(batch_ids, kind="stable")
        feat = feat[perm]
        batch_ids = batch_ids[perm]

    counts = np.bincount(batch_ids, minlength=N_INSTANCES).astype(np.int64)
    starts = np.zeros(N_INSTANCES + 1, dtype=np.int64)
    np.cumsum(counts, out=starts[1:])

    L = int(max(512, ((counts.max() + 511) // 512) * 512))
    nc = _get_program(L)

    g4 = np.tile(np.eye(C, dtype=np.float32), (4, 1))
    dmask = np.eye(P, dtype=np.float32)

    in_maps = []
    for k in range(NCORES):
        x_pad = np.zeros((IPC, L, C), dtype=np.float32)
        cinv = np.zeros((1, IPC), dtype=np.float32)
        for j in range(IPC):
            inst = k * IPC + j
            s, e = starts[inst], starts[inst + 1]
            x_pad[j, : e - s] = feat[s:e]
            cinv[0, j] = 1.0 / max(float(counts[inst]), 1.0)
        in_maps.append(
            {
                "x": x_pad,
                "wvec": weight,
                "bvec": bias,
                "cinv": cinv,
                "g4": g4,
                "dmask": dmask,
            }
        )

    res = run_bass_kernel_spmd(
        nc,
        in_maps,
        list(range(NCORES)),
        trace=trace,
        **(trace_kwargs or {}),
    )

    out = np.empty((n, C), dtype=np.float32)
    for k in range(NCORES):
        y = res.results[k]["y"]
        for j in range(IPC):
            inst = k * IPC + j
            s, e = starts[inst], starts[inst + 1]
            out[s:e] = y[j, : e - s]

    if perm is not None:
        inv = np.empty_like(perm)
        inv[perm] = np.arange(n)
        out = out[inv]
    return out, res


def kernel(feat, batch_ids, weight, bias):
    out, _ = _run(feat, batch_ids, weight, bias, trace=False)
    return out


# revision 17
# speedup vs baseline: 1.0151x; 1.0151x over previous
"""MinkowskiInstanceNorm on 8 Trainium2 NeuronCores.

Strategy: batch_ids are sorted, so the 16 instances are contiguous row
ranges.  Assign 2 instances per core (no cross-core collectives).  On
host, pad each instance to a uniform length L (zeros) so the SPMD
program is identical across cores.  Per instance on device:
  pass 1: DMA rows into SBUF fp32 panels (each partition holds a
          contiguous row range -> fully contiguous HBM transfers).
          ScalarE casts each panel to a bf16 shadow with a ones column
          per 128-col wide tile; TensorE then computes Gram + sums in a
          single cheap bf16 matmul per wide tile, accumulated in PSUM
          (diag = per-(group,channel) sum of squares, ones col = sums).
          Zero padding contributes nothing.
  stats:  extract diag + sums, fold the 4 row groups per channel with
          two tiny matmuls, compute A = weight/sqrt(var+eps),
          B = bias - mean*A, broadcast to [128, 128] via a K=1 matmul.
  pass 2: y = x*A + B with two VectorE tensor_tensor ops per panel
          (in place on the fp32 panel), then DMA out.  Rows are read
          from HBM once and written once (~64 MB per core).
"""

import numpy as np

import concourse.bass as bass
import concourse.mybir as mybir
import concourse.tile as tile
import concourse.tile_utils as tile_utils
from concourse.alu_op_type import AluOpType
from concourse.bass_utils import run_bass_kernel_spmd

F32 = mybir.dt.float32
BF16 = mybir.dt.bfloat16
P = 128            # SBUF partitions
C = 32             # channels
IPC = 2            # instances per core
NCORES = 8
N_INSTANCES = 16
EPS = 1e-8
ROWS_PP = 16384    # rows per full panel (128 rows per partition, 2 MiB)

# use the full usable SBUF (the default cap leaves 16 KiB/partition unused)
tile_utils.max_sbuf_usage = 208 * 1024

_PROG_CACHE: dict = {}


def _build_program(L: int):
    """Build the SPMD bass program for instance slot length L (mult of 512)."""
    n_panels = (L + ROWS_PP - 1) // ROWS_PP
    nc = bass.Bass()

    x_ext = nc.declare_dram_parameter("x", [IPC, L, C], F32, isOutput=False)
    w_ext = nc.declare_dram_parameter("wvec", [1, C], F32, isOutput=False)
    b_ext = nc.declare_dram_parameter("bvec", [1, C], F32, isOutput=False)
    cinv_ext = nc.declare_dram_parameter("cinv", [1, IPC], F32, isOutput=False)
    g4_ext = nc.declare_dram_parameter("g4", [P, C], F32, isOutput=False)
    dm_ext = nc.declare_dram_parameter("dmask", [P, P], F32, isOutput=False)
    y_ext = nc.declare_dram_parameter("y", [IPC, L, C], F32, isOutput=True)

    with tile.TileContext(nc) as tc:
        with (
            tc.tile_pool(name="panels", bufs=11) as panels,
            tc.tile_pool(name="bfp", bufs=1) as bfp,
            tc.tile_pool(name="consts", bufs=1) as consts,
            tc.tile_pool(name="small", bufs=2) as small,
            tc.tile_pool(name="psum_acc", bufs=2, space="PSUM") as psum_acc,
            tc.tile_pool(name="psum_sm", bufs=1, space="PSUM") as psum_sm,
        ):
            w_sb = consts.tile([1, C], F32, tag="w")
            nc.sync.dma_start(out=w_sb[:], in_=w_ext[:])
            b_sb = consts.tile([1, C], F32, tag="b")
            nc.sync.dma_start(out=b_sb[:], in_=b_ext[:])
            cinv_sb = consts.tile([1, IPC], F32, tag="cinv")
            nc.sync.dma_start(out=cinv_sb[:], in_=cinv_ext[:])
            g4_sb = consts.tile([P, C], F32, tag="g4")
            nc.sync.dma_start(out=g4_sb[:], in_=g4_ext[:])
            dm_sb = consts.tile([P, P], F32, tag="dmask")
            nc.sync.dma_start(out=dm_sb[:], in_=dm_ext[:])
            ones_row = consts.tile([1, P], F32, tag="ones_row")
            nc.vector.memset(ones_row[:], 1.0)
            eps_sb = consts.tile([1, 1], F32, tag="eps")
            nc.vector.memset(eps_sb[:], float(EPS))

            # 4 persistent bf16 shadow tiles (double-buffered by WAR deps);
            # ones columns are written once and never overwritten
            CH = 16   # wide tiles per cast chunk (half panel)
            NBF = 4
            bf_tiles = []
            for t in range(NBF):
                bt = consts.tile([P, CH * 129], BF16, tag=f"bf{t}")
                bt3 = bt[:].rearrange("q (n k) -> q n k", k=129)
                nc.vector.memset(bt3[:, :, 128:129], 1.0)
                bf_tiles.append(bt)
            chunk_ctr = [0]

            def pass1(i):
                # load, cast to bf16, accumulate Gram+sums on PE
                acc_ps = psum_acc.tile([P, P + 1], F32, tag="acc")
                ptiles = []
                total_wt = L // 512
                wt_done = 0
                for p in range(n_panels):
                    r0 = p * ROWS_PP
                    rows = min(ROWS_PP, L - r0)
                    rpp = rows // P       # rows per partition
                    nwt = rpp // 4        # wide tiles (128 cols each)
                    pt = panels.tile([P, (ROWS_PP // P) * C], F32, tag="panel")
                    src = x_ext[i, r0 : r0 + rows, :].rearrange(
                        "(q n) c -> q (n c)", q=P
                    )
                    nc.gpsimd.dma_start(out=pt[:, : rpp * C], in_=src)
                    ptiles.append((pt, r0, rows, rpp, nwt))

                    pt3 = pt[:].rearrange("q (n k) -> q n k", k=P)
                    for c0 in range(0, nwt, CH):
                        cw = min(CH, nwt - c0)
                        bt = bf_tiles[chunk_ctr[0] % NBF]
                        chunk_ctr[0] += 1
                        bt3 = bt[:].rearrange("q (n k) -> q n k", k=129)
                        nc.scalar.copy(
                            bt3[:, :cw, 0:P], pt3[:, c0 : c0 + cw, :]
                        )
                        for wt in range(cw):
                            nc.tensor.matmul(
                                acc_ps[:],
                                bt[:, wt * 129 : wt * 129 + P],
                                bt[:, wt * 129 : wt * 129 + P + 1],
                                start=(wt_done == 0),
                                stop=(wt_done == total_wt - 1),
                            )
                            wt_done += 1
                assert wt_done == total_wt
                return acc_ps, ptiles

            def stats(i, acc_ps):
                ds_sb = small.tile([P, 2], F32, tag="ds")
                scratch = psum_sm.tile([P, P], F32, tag="scratch")
                nc.vector.tensor_tensor(
                    scratch[:], acc_ps[:, 0:P], dm_sb[:], AluOpType.mult
                )
                nc.vector.tensor_reduce(
                    ds_sb[:, 0:1], scratch[:], mybir.AxisListType.X, AluOpType.add
                )
                nc.vector.tensor_copy(ds_sb[:, 1:2], acc_ps[:, P : P + 1])

                d_ps = psum_sm.tile([1, C], F32, tag="d")
                s_ps = psum_sm.tile([1, C], F32, tag="s")
                nc.tensor.matmul(
                    d_ps[:], ds_sb[:, 0:1], g4_sb[:], start=True, stop=True
                )
                nc.tensor.matmul(
                    s_ps[:], ds_sb[:, 1:2], g4_sb[:], start=True, stop=True
                )

                cinv_i = cinv_sb[0:1, i : i + 1]
                mean_sb = small.tile([1, C], F32, tag="mean")
                ex2_sb = small.tile([1, C], F32, tag="ex2")
                var_sb = small.tile([1, C], F32, tag="var")
                std_sb = small.tile([1, C], F32, tag="std")
                istd_sb = small.tile([1, C], F32, tag="istd")
                tmp_sb = small.tile([1, C], F32, tag="tmp")
                ab4_sb = small.tile([1, 8 * C], F32, tag="ab4")

                nc.vector.tensor_scalar_mul(mean_sb[:], s_ps[:], cinv_i)
                nc.vector.tensor_scalar_mul(ex2_sb[:], d_ps[:], cinv_i)
                nc.vector.tensor_mul(var_sb[:], mean_sb[:], mean_sb[:])
                nc.vector.tensor_sub(var_sb[:], ex2_sb[:], var_sb[:])
                nc.scalar.activation(
                    std_sb[:],
                    var_sb[:],
                    mybir.ActivationFunctionType.Sqrt,
                    bias=eps_sb[:],
                )
                nc.vector.reciprocal(istd_sb[:], std_sb[:])
                nc.vector.tensor_mul(ab4_sb[:, 0:C], istd_sb[:], w_sb[:])
                nc.vector.tensor_mul(tmp_sb[:], mean_sb[:], ab4_sb[:, 0:C])
                nc.vector.tensor_sub(ab4_sb[:, 4 * C : 5 * C], b_sb[:], tmp_sb[:])
                # replicate A to slots 1..3 and B to slots 5..7
                nc.vector.tensor_copy(ab4_sb[:, C : 2 * C], ab4_sb[:, 0:C])
                nc.vector.tensor_copy(ab4_sb[:, 2 * C : 4 * C], ab4_sb[:, 0 : 2 * C])
                nc.vector.tensor_copy(
                    ab4_sb[:, 5 * C : 6 * C], ab4_sb[:, 4 * C : 5 * C]
                )
                nc.vector.tensor_copy(
                    ab4_sb[:, 6 * C : 8 * C], ab4_sb[:, 4 * C : 6 * C]
                )

                # broadcast [1, 256] -> [128, 256] via K=1 matmul
                ab_ps = psum_sm.tile([P, 8 * C], F32, tag="abps")
                nc.tensor.matmul(
                    ab_ps[:], ones_row[:], ab4_sb[:], start=True, stop=True
                )
                ab_rep = small.tile([P, 8 * C], F32, tag="abrep")
                nc.scalar.copy(ab_rep[:], ab_ps[:])
                return ab_rep

            def pass2(i, ptiles, ab_rep):
                # normalize in place + store
                a_wide = ab_rep[:, 0 : 4 * C]        # [128, 128]
                b_wide = ab_rep[:, 4 * C : 8 * C]    # [128, 128]
                for pt, r0, rows, rpp, nwt in ptiles:
                    pv = pt[:].rearrange("q (n k) -> q n k", k=P)[:, :nwt, :]
                    a_b = a_wide[:, None, :].broadcast_to([P, nwt, P])
                    b_b = b_wide[:, None, :].broadcast_to([P, nwt, P])
                    nc.vector.tensor_tensor(pv, pv, a_b, AluOpType.mult)
                    nc.vector.tensor_tensor(pv, pv, b_b, AluOpType.add)
                    dst = y_ext[i, r0 : r0 + rows, :].rearrange(
                        "(q n) c -> q (n c)", q=P
                    )
                    nc.sync.dma_start(out=dst, in_=pt[:, : rpp * C])

            # phase order: instance 1's pass-1 is emitted before instance
            # 0's normalize so its loads/casts/matmuls queue ahead and fill
            # the DMA/PE while VectorE runs instance 0's tensor_tensor ops.
            acc0, pt0 = pass1(0)
            ab0 = stats(0, acc0)
            acc1, pt1 = pass1(1)
            pass2(0, pt0, ab0)
            ab1 = stats(1, acc1)
            pass2(1, pt1, ab1)

    # Populate .instr bytes for extended-inst InstISA subclasses — raw bass
    # skips this pass and the NEFF compiler fails with "ISA wrong length"
    # on empty .instr.
    mybir.codegen_inst_isa_subclasses(nc)
    _split_waits(nc)
    return nc


def _split_waits(nc, max_waits: int = 1):
    """This container's walrus rejects instructions carrying more than one
    semaphore wait ("Too many sync wait commands").  Hoist extra waits onto
    same-engine InstNoOps inserted just before the instruction.
    """
    for f in nc.m.functions:
        for blk in f.blocks:
            new = []
            for inst in blk.instructions:
                si = inst.sync_info
                if (
                    si is not None
                    and len(si.on_wait) > max_waits
                    and not isinstance(inst, mybir.InstNoOp)
                ):
                    waits = list(si.on_wait)
                    for w in waits[:-max_waits]:
                        nop = mybir.InstNoOp(
                            name=nc.get_next_instruction_name(),
                            engine=inst.engine,
                            sync_info=mybir.SyncInfo(on_wait=[w], on_update=[]),
                            bass_nofuse=True,
                        )
                        new.append(nop)
                    inst.sync_info = mybir.SyncInfo(
                        on_wait=waits[-max_waits:], on_update=list(si.on_update)
                    )
                new.append(inst)
            blk.instructions[:] = new


def _get_program(L: int):
    prog = _PROG_CACHE.get(L)
    if prog is None:
        prog = _build_program(L)
        _PROG_CACHE[L] = prog
    return prog


def _run(feat, batch_ids, weight, bias, trace=False, trace_kwargs=None):
    feat = np.ascontiguousarray(np.asarray(feat, dtype=np.float32))
    batch_ids = np.asarray(batch_ids, dtype=np.int32)
    weight = np.asarray(weight, dtype=np.float32).reshape(1, C)
    bias = np.asarray(bias, dtype=np.float32).reshape(1, C)
    n = feat.shape[0]

    perm = None
    if np.any(np.diff(batch_ids) < 0):  # insurance; spec says sorted
        perm = np.argsort(batch_ids, kind="stable")
        feat = feat[perm]
        batch_ids = batch_ids[perm]

    counts = np.bincount(batch_ids, minlength=N_INSTANCES).astype(np.int64)
    starts = np.zeros(N_INSTANCES + 1, dtype=np.int64)
    np.cumsum(counts, out=starts[1:])

    L = int(max(512, ((counts.max() + 511) // 512) * 512))
    nc = _get_program(L)

    g4 = np.tile(np.eye(C, dtype=np.float32), (4, 1))
    dmask = np.eye(P, dtype=np.float32)

    in_maps = []
    for k in range(NCORES):
        x_pad = np.zeros((IPC, L, C), dtype=np.float32)
        cinv = np.zeros((1, IPC), dtype=np.float32)
        for j in range(IPC):
            inst = k * IPC + j
            s, e = starts[inst], starts[inst + 1]
            x_pad[j, : e - s] = feat[s:e]
            cinv[0, j] = 1.0 / max(float(counts[inst]), 1.0)
        in_maps.append(
            {
                "x": x_pad,
                "wvec": weight,
                "bvec": bias,
                "cinv": cinv,
                "g4": g4,
                "dmask": dmask,
            }
        )

    res = run_bass_kernel_spmd(
        nc,
        in_maps,
        list(range(NCORES)),
        trace=trace,
        **(trace_kwargs or {}),
    )

    out = np.empty((n, C), dtype=np.float32)
    for k in range(NCORES):
        y = res.results[k]["y"]
        for j in range(IPC):
            inst = k * IPC + j
            s, e = starts[inst], starts[inst + 1]
            out[s:e] = y[j, : e - s]

    if perm is not None:
        inv = np.empty_like(perm)
        inv[perm] = np.arange(n)
        out = out[inv]
    return out, res


def kernel(feat, batch_ids, weight, bias):
    out, _ = _run(feat, batch_ids, weight, bias, trace=False)
    return out


# revision 18
# speedup vs baseline: 1.0884x; 1.0722x over previous
"""MinkowskiInstanceNorm on 8 Trainium2 NeuronCores.

Strategy: batch_ids are sorted, so the 16 instances are contiguous row
ranges.  Assign 2 instances per core (no cross-core collectives).  On
host, pad each instance to a uniform length L (zeros) so the SPMD
program is identical across cores.  Per instance on device:
  pass 1: DMA rows into SBUF fp32 panels (each partition holds a
          contiguous row range -> fully contiguous HBM transfers).
          ScalarE casts each panel to a bf16 shadow with a ones column
          per 128-col wide tile; TensorE then computes Gram + sums in a
          single cheap bf16 matmul per wide tile, accumulated in PSUM
          (diag = per-(group,channel) sum of squares, ones col = sums).
          Zero padding contributes nothing.
  stats:  extract diag + sums, fold the 4 row groups per channel with
          two tiny matmuls, compute A = weight/sqrt(var+eps),
          B = bias - mean*A, broadcast to [128, 128] via a K=1 matmul.
  pass 2: y = x*A + B with two VectorE tensor_tensor ops per panel
          (in place on the fp32 panel), then DMA out.  Rows are read
          from HBM once and written once (~64 MB per core).
"""

import numpy as np

import concourse.bass as bass
import concourse.mybir as mybir
import concourse.tile as tile
import concourse.tile_utils as tile_utils
from concourse.alu_op_type import AluOpType
from concourse.bass_utils import run_bass_kernel_spmd

F32 = mybir.dt.float32
BF16 = mybir.dt.bfloat16
P = 128            # SBUF partitions
C = 32             # channels
IPC = 2            # instances per core
NCORES = 8
N_INSTANCES = 16
EPS = 1e-8
ROWS_PP = 16384    # rows per full panel (128 rows per partition, 2 MiB)

# use the full usable SBUF (the default cap leaves 16 KiB/partition unused)
tile_utils.max_sbuf_usage = 208 * 1024

_PROG_CACHE: dict = {}


def _build_program(L: int):
    """Build the SPMD bass program for instance slot length L (mult of 512)."""
    n_panels = (L + ROWS_PP - 1) // ROWS_PP
    nc = bass.Bass()

    x_ext = nc.declare_dram_parameter("x", [IPC, L, C], F32, isOutput=False)
    w_ext = nc.declare_dram_parameter("wvec", [1, C], F32, isOutput=False)
    b_ext = nc.declare_dram_parameter("bvec", [1, C], F32, isOutput=False)
    cinv_ext = nc.declare_dram_parameter("cinv", [1, IPC], F32, isOutput=False)
    g4_ext = nc.declare_dram_parameter("g4", [P, C], F32, isOutput=False)
    dm_ext = nc.declare_dram_parameter("dmask", [P, P], F32, isOutput=False)
    y_ext = nc.declare_dram_parameter("y", [IPC, L, C], F32, isOutput=True)

    with tile.TileContext(nc) as tc:
        with (
            tc.tile_pool(name="panels", bufs=11) as panels,
            tc.tile_pool(name="bfp", bufs=1) as bfp,
            tc.tile_pool(name="consts", bufs=1) as consts,
            tc.tile_pool(name="small", bufs=2) as small,
            tc.tile_pool(name="psum_acc", bufs=2, space="PSUM") as psum_acc,
            tc.tile_pool(name="psum_sm", bufs=1, space="PSUM") as psum_sm,
        ):
            w_sb = consts.tile([1, C], F32, tag="w")
            nc.sync.dma_start(out=w_sb[:], in_=w_ext[:])
            b_sb = consts.tile([1, C], F32, tag="b")
            nc.sync.dma_start(out=b_sb[:], in_=b_ext[:])
            cinv_sb = consts.tile([1, IPC], F32, tag="cinv")
            nc.sync.dma_start(out=cinv_sb[:], in_=cinv_ext[:])
            g4_sb = consts.tile([P, C], F32, tag="g4")
            nc.sync.dma_start(out=g4_sb[:], in_=g4_ext[:])
            dm_sb = consts.tile([P, P], F32, tag="dmask")
            nc.sync.dma_start(out=dm_sb[:], in_=dm_ext[:])
            ones_row = consts.tile([1, P], F32, tag="ones_row")
            nc.vector.memset(ones_row[:], 1.0)
            eps_sb = consts.tile([1, 1], F32, tag="eps")
            nc.vector.memset(eps_sb[:], float(EPS))

            # 4 persistent bf16 shadow tiles (double-buffered by WAR deps);
            # ones columns are written once and never overwritten
            CH = 16   # wide tiles per cast chunk (half panel)
            NBF = 4
            bf_tiles = []
            for t in range(NBF):
                bt = consts.tile([P, CH * 130], BF16, tag=f"bf{t}")
                bt3 = bt[:].rearrange("q (n k) -> q n k", k=130)
                nc.vector.memset(bt3[:, :, 128:129], 1.0)
                bf_tiles.append(bt)
            chunk_ctr = [0]

            def pass1(i, cast):
                # load, cast to bf16, accumulate Gram+sums on PE
                acc_ps = psum_acc.tile([P, P + 1], F32, tag="acc")
                ptiles = []
                total_wt = L // 512
                wt_done = 0
                for p in range(n_panels):
                    r0 = p * ROWS_PP
                    rows = min(ROWS_PP, L - r0)
                    rpp = rows // P       # rows per partition
                    nwt = rpp // 4        # wide tiles (128 cols each)
                    pt = panels.tile([P, (ROWS_PP // P) * C], F32, tag="panel")
                    src = x_ext[i, r0 : r0 + rows, :].rearrange(
                        "(q n) c -> q (n c)", q=P
                    )
                    nc.gpsimd.dma_start(out=pt[:, : rpp * C], in_=src)
                    ptiles.append((pt, r0, rows, rpp, nwt))

                    pt3 = pt[:].rearrange("q (n k) -> q n k", k=P)
                    for c0 in range(0, nwt, CH):
                        cw = min(CH, nwt - c0)
                        bt = bf_tiles[chunk_ctr[0] % NBF]
                        chunk_ctr[0] += 1
                        bt3 = bt[:].rearrange("q (n k) -> q n k", k=130)
                        cast(bt3[:, :cw, 0:P], pt3[:, c0 : c0 + cw, :])
                        for wt in range(cw):
                            nc.tensor.matmul(
                                acc_ps[:],
                                bt[:, wt * 130 : wt * 130 + P],
                                bt[:, wt * 130 : wt * 130 + P + 1],
                                start=(wt_done == 0),
                                stop=(wt_done == total_wt - 1),
                            )
                            wt_done += 1
                assert wt_done == total_wt
                return acc_ps, ptiles

            def stats(i, acc_ps):
                ds_sb = small.tile([P, 2], F32, tag="ds")
                scratch = psum_sm.tile([P, P], F32, tag="scratch")
                nc.vector.tensor_tensor(
                    scratch[:], acc_ps[:, 0:P], dm_sb[:], AluOpType.mult
                )
                nc.vector.tensor_reduce(
                    ds_sb[:, 0:1], scratch[:], mybir.AxisListType.X, AluOpType.add
                )
                nc.vector.tensor_copy(ds_sb[:, 1:2], acc_ps[:, P : P + 1])

                d_ps = psum_sm.tile([1, C], F32, tag="d")
                s_ps = psum_sm.tile([1, C], F32, tag="s")
                nc.tensor.matmul(
                    d_ps[:], ds_sb[:, 0:1], g4_sb[:], start=True, stop=True
                )
                nc.tensor.matmul(
                    s_ps[:], ds_sb[:, 1:2], g4_sb[:], start=True, stop=True
                )

                cinv_i = cinv_sb[0:1, i : i + 1]
                mean_sb = small.tile([1, C], F32, tag="mean")
                ex2_sb = small.tile([1, C], F32, tag="ex2")
                var_sb = small.tile([1, C], F32, tag="var")
                std_sb = small.tile([1, C], F32, tag="std")
                istd_sb = small.tile([1, C], F32, tag="istd")
                tmp_sb = small.tile([1, C], F32, tag="tmp")
                ab4_sb = small.tile([1, 8 * C], F32, tag="ab4")

                nc.vector.tensor_scalar_mul(mean_sb[:], s_ps[:], cinv_i)
                nc.vector.tensor_scalar_mul(ex2_sb[:], d_ps[:], cinv_i)
                nc.vector.tensor_mul(var_sb[:], mean_sb[:], mean_sb[:])
                nc.vector.tensor_sub(var_sb[:], ex2_sb[:], var_sb[:])
                nc.scalar.activation(
                    std_sb[:],
                    var_sb[:],
                    mybir.ActivationFunctionType.Sqrt,
                    bias=eps_sb[:],
                )
                nc.vector.reciprocal(istd_sb[:], std_sb[:])
                nc.vector.tensor_mul(ab4_sb[:, 0:C], istd_sb[:], w_sb[:])
                nc.vector.tensor_mul(tmp_sb[:], mean_sb[:], ab4_sb[:, 0:C])
                nc.vector.tensor_sub(ab4_sb[:, 4 * C : 5 * C], b_sb[:], tmp_sb[:])
                # replicate A to slots 1..3 and B to slots 5..7
                nc.vector.tensor_copy(ab4_sb[:, C : 2 * C], ab4_sb[:, 0:C])
                nc.vector.tensor_copy(ab4_sb[:, 2 * C : 4 * C], ab4_sb[:, 0 : 2 * C])
                nc.vector.tensor_copy(
                    ab4_sb[:, 5 * C : 6 * C], ab4_sb[:, 4 * C : 5 * C]
                )
                nc.vector.tensor_copy(
                    ab4_sb[:, 6 * C : 8 * C], ab4_sb[:, 4 * C : 6 * C]
                )

                # broadcast [1, 256] -> [128, 256] via K=1 matmul
                ab_ps = psum_sm.tile([P, 8 * C], F32, tag="abps")
                nc.tensor.matmul(
                    ab_ps[:], ones_row[:], ab4_sb[:], start=True, stop=True
                )
                ab_rep = small.tile([P, 8 * C], F32, tag="abrep")
                nc.scalar.copy(ab_rep[:], ab_ps[:])
                return ab_rep

            def pass2(i, ptiles, ab_rep):
                # normalize in place + store
                a_wide = ab_rep[:, 0 : 4 * C]        # [128, 128]
                b_wide = ab_rep[:, 4 * C : 8 * C]    # [128, 128]
                for pt, r0, rows, rpp, nwt in ptiles:
                    pv = pt[:].rearrange("q (n k) -> q n k", k=P)[:, :nwt, :]
                    a_b = a_wide[:, None, :].broadcast_to([P, nwt, P])
                    b_b = b_wide[:, None, :].broadcast_to([P, nwt, P])
                    nc.vector.tensor_tensor(pv, pv, a_b, AluOpType.mult)
                    nc.vector.tensor_tensor(pv, pv, b_b, AluOpType.add)
                    dst = y_ext[i, r0 : r0 + rows, :].rearrange(
                        "(q n) c -> q (n c)", q=P
                    )
                    nc.sync.dma_start(out=dst, in_=pt[:, : rpp * C])

            # phase order: instance 1's pass-1 is emitted before instance
            # 0's normalize so its loads/casts/matmuls queue ahead and fill
            # the DMA/PE while VectorE runs instance 0's tensor_tensor ops.
            acc0, pt0 = pass1(0, nc.vector.tensor_copy)
            ab0 = stats(0, acc0)
            acc1, pt1 = pass1(1, nc.scalar.copy)
            pass2(0, pt0, ab0)
            ab1 = stats(1, acc1)
            pass2(1, pt1, ab1)

    # Populate .instr bytes for extended-inst InstISA subclasses — raw bass
    # skips this pass and the NEFF compiler fails with "ISA wrong length"
    # on empty .instr.
    mybir.codegen_inst_isa_subclasses(nc)
    _split_waits(nc)
    return nc


def _split_waits(nc, max_waits: int = 1):
    """This container's walrus rejects instructions carrying more than one
    semaphore wait ("Too many sync wait commands").  Hoist extra waits onto
    same-engine InstNoOps inserted just before the instruction.
    """
    for f in nc.m.functions:
        for blk in f.blocks:
            new = []
            for inst in blk.instructions:
                si = inst.sync_info
                if (
                    si is not None
                    and len(si.on_wait) > max_waits
                    and not isinstance(inst, mybir.InstNoOp)
                ):
                    waits = list(si.on_wait)
                    for w in waits[:-max_waits]:
                        nop = mybir.InstNoOp(
                            name=nc.get_next_instruction_name(),
                            engine=inst.engine,
                            sync_info=mybir.SyncInfo(on_wait=[w], on_update=[]),
                            bass_nofuse=True,
                        )
                        new.append(nop)
                    inst.sync_info = mybir.SyncInfo(
                        on_wait=waits[-max_waits:], on_update=list(si.on_update)
                    )
                new.append(inst)
            blk.instructions[:] = new


def _get_program(L: int):
    prog = _PROG_CACHE.get(L)
    if prog is None:
        prog = _build_program(L)
        _PROG_CACHE[L] = prog
    return prog


def _run(feat, batch_ids, weight, bias, trace=False, trace_kwargs=None):
    feat = np.ascontiguousarray(np.asarray(feat, dtype=np.float32))
    batch_ids = np.asarray(batch_ids, dtype=np.int32)
    weight = np.asarray(weight, dtype=np.float32).reshape(1, C)
    bias = np.asarray(bias, dtype=np.float32).reshape(1, C)
    n = feat.shape[0]

    perm = None
    if np.any(np.diff(batch_ids) < 0):  # insurance; spec says sorted
        perm = np.argsort(batch_ids, kind="stable")
        feat = feat[perm]
        batch_ids = batch_ids[perm]

    counts = np.bincount(batch_ids, minlength=N_INSTANCES).astype(np.int64)
    starts = np.zeros(N_INSTANCES + 1, dtype=np.int64)
    np.cumsum(counts, out=starts[1:])

    L = int(max(512, ((counts.max() + 511) // 512) * 512))
    nc = _get_program(L)

    g4 = np.tile(np.eye(C, dtype=np.float32), (4, 1))
    dmask = np.eye(P, dtype=np.float32)

    in_maps = []
    for k in range(NCORES):
        x_pad = np.zeros((IPC, L, C), dtype=np.float32)
        cinv = np.zeros((1, IPC), dtype=np.float32)
        for j in range(IPC):
            inst = k * IPC + j
            s, e = starts[inst], starts[inst + 1]
            x_pad[j, : e - s] = feat[s:e]
            cinv[0, j] = 1.0 / max(float(counts[inst]), 1.0)
        in_maps.append(
            {
                "x": x_pad,
                "wvec": weight,
                "bvec": bias,
                "cinv": cinv,
                "g4": g4,
                "dmask": dmask,
            }
        )

    res = run_bass_kernel_spmd(
        nc,
        in_maps,
        list(range(NCORES)),
        trace=trace,
        **(trace_kwargs or {}),
    )

    out = np.empty((n, C), dtype=np.float32)
    for k in range(NCORES):
        y = res.results[k]["y"]
        for j in range(IPC):
            inst = k * IPC + j
            s, e = starts[inst], starts[inst + 1]
            out[s:e] = y[j, : e - s]

    if perm is not None:
        inv = np.empty_like(perm)
        inv[perm] = np.arange(n)
        out = out[inv]
    return out, res


def kernel(feat, batch_ids, weight, bias):
    out, _ = _run(feat, batch_ids, weight, bias, trace=False)
    return out


# revision 22
# speedup vs baseline: 1.1072x; 1.0173x over previous
"""MinkowskiInstanceNorm on 8 Trainium2 NeuronCores.

Strategy: batch_ids are sorted, so the 16 instances are contiguous row
ranges.  Assign 2 instances per core (no cross-core collectives).  On
host, pad each instance to a uniform length L (zeros) so the SPMD
program is identical across cores.  Per instance on device:
  pass 1: DMA rows into SBUF fp32 panels (each partition holds a
          contiguous row range -> fully contiguous HBM transfers).
          ScalarE casts each panel to a bf16 shadow with a ones column
          per 128-col wide tile; TensorE then computes Gram + sums in a
          single cheap bf16 matmul per wide tile, accumulated in PSUM
          (diag = per-(group,channel) sum of squares, ones col = sums).
          Zero padding contributes nothing.
  stats:  extract diag + sums, fold the 4 row groups per channel with
          two tiny matmuls, compute A = weight/sqrt(var+eps),
          B = bias - mean*A, broadcast to [128, 128] via a K=1 matmul.
  pass 2: y = x*A + B with two VectorE tensor_tensor ops per panel
          (in place on the fp32 panel), then DMA out.  Rows are read
          from HBM once and written once (~64 MB per core).
"""

import numpy as np

import concourse.bass as bass
import concourse.mybir as mybir
import concourse.tile as tile
import concourse.tile_utils as tile_utils
from concourse.alu_op_type import AluOpType
from concourse.bass_utils import run_bass_kernel_spmd
from concourse.tile_rust import add_dep_helper

F32 = mybir.dt.float32
BF16 = mybir.dt.bfloat16
P = 128            # SBUF partitions
C = 32             # channels
IPC = 2            # instances per core
NCORES = 8
N_INSTANCES = 16
EPS = 1e-8
ROWS_PP = 16384    # rows per full panel (128 rows per partition, 2 MiB)

# use the full usable SBUF (the default cap leaves 16 KiB/partition unused)
tile_utils.max_sbuf_usage = 208 * 1024

_PROG_CACHE: dict = {}


def _build_program(L: int):
    """Build the SPMD bass program for instance slot length L (mult of 512)."""
    n_panels = (L + ROWS_PP - 1) // ROWS_PP
    nc = bass.Bass()

    x_ext = nc.declare_dram_parameter("x", [IPC, L, C], F32, isOutput=False)
    w_ext = nc.declare_dram_parameter("wvec", [1, C], F32, isOutput=False)
    b_ext = nc.declare_dram_parameter("bvec", [1, C], F32, isOutput=False)
    cinv_ext = nc.declare_dram_parameter("cinv", [1, IPC], F32, isOutput=False)
    g4_ext = nc.declare_dram_parameter("g4", [P, C], F32, isOutput=False)
    dm_ext = nc.declare_dram_parameter("dmask", [P, P], F32, isOutput=False)
    y_ext = nc.declare_dram_parameter("y", [IPC, L, C], F32, isOutput=True)

    with tile.TileContext(nc) as tc:
        with (
            tc.tile_pool(name="panels", bufs=11) as panels,
            tc.tile_pool(name="bfp", bufs=1) as bfp,
            tc.tile_pool(name="consts", bufs=1) as consts,
            tc.tile_pool(name="small", bufs=2) as small,
            tc.tile_pool(name="psum_acc", bufs=2, space="PSUM") as psum_acc,
            tc.tile_pool(name="psum_sm", bufs=1, space="PSUM") as psum_sm,
        ):
            w_sb = consts.tile([1, C], F32, tag="w")
            nc.sync.dma_start(out=w_sb[:], in_=w_ext[:])
            b_sb = consts.tile([1, C], F32, tag="b")
            nc.sync.dma_start(out=b_sb[:], in_=b_ext[:])
            cinv_sb = consts.tile([1, IPC], F32, tag="cinv")
            nc.sync.dma_start(out=cinv_sb[:], in_=cinv_ext[:])
            g4_sb = consts.tile([P, C], F32, tag="g4")
            nc.sync.dma_start(out=g4_sb[:], in_=g4_ext[:])
            dm_sb = consts.tile([P, P], F32, tag="dmask")
            nc.sync.dma_start(out=dm_sb[:], in_=dm_ext[:])
            ones_row = consts.tile([1, P], F32, tag="ones_row")
            nc.vector.memset(ones_row[:], 1.0)
            eps_sb = consts.tile([1, 1], F32, tag="eps")
            nc.vector.memset(eps_sb[:], float(EPS))

            # 4 persistent bf16 shadow tiles (double-buffered by WAR deps);
            # ones columns are written once and never overwritten
            CH = 16   # wide tiles per cast chunk (half panel)
            NBF = 4
            bf_tiles = []
            for t in range(NBF):
                bt = consts.tile([P, CH * 130], BF16, tag=f"bf{t}")
                bt3 = bt[:].rearrange("q (n k) -> q n k", k=130)
                nc.vector.memset(bt3[:, :, 128:129], 1.0)
                bf_tiles.append(bt)
            chunk_ctr = [0]

            def pass1(i, cast):
                # load, cast to bf16, accumulate Gram+sums on PE
                acc_ps = psum_acc.tile([P, P + 1], F32, tag="acc")
                ptiles = []
                total_wt = L // 512
                wt_done = 0
                for p in range(n_panels):
                    r0 = p * ROWS_PP
                    rows = min(ROWS_PP, L - r0)
                    rpp = rows // P       # rows per partition
                    nwt = rpp // 4        # wide tiles (128 cols each)
                    pt = panels.tile([P, (ROWS_PP // P) * C], F32, tag="panel")
                    src = x_ext[i, r0 : r0 + rows, :].rearrange(
                        "(q n) c -> q (n c)", q=P
                    )
                    ld = nc.gpsimd.dma_start(out=pt[:, : rpp * C], in_=src)
                    ptiles.append((pt, r0, rows, rpp, nwt))
                    last_load[0] = ld.ins

                    pt3 = pt[:].rearrange("q (n k) -> q n k", k=P)
                    for c0 in range(0, nwt, CH):
                        cw = min(CH, nwt - c0)
                        bt = bf_tiles[chunk_ctr[0] % NBF]
                        chunk_ctr[0] += 1
                        bt3 = bt[:].rearrange("q (n k) -> q n k", k=130)
                        cast(bt3[:, :cw, 0:P], pt3[:, c0 : c0 + cw, :])
                        for wt in range(cw):
                            nc.tensor.matmul(
                                acc_ps[:],
                                bt[:, wt * 130 : wt * 130 + P],
                                bt[:, wt * 130 : wt * 130 + P + 1],
                                start=(wt_done == 0),
                                stop=(wt_done == total_wt - 1),
                            )
                            wt_done += 1
                assert wt_done == total_wt
                return acc_ps, ptiles

            def stats(i, acc_ps):
                ds_sb = small.tile([P, 2], F32, tag="ds")
                scratch = psum_sm.tile([P, P], F32, tag="scratch")
                nc.vector.tensor_tensor(
                    scratch[:], acc_ps[:, 0:P], dm_sb[:], AluOpType.mult
                )
                nc.vector.tensor_reduce(
                    ds_sb[:, 0:1], scratch[:], mybir.AxisListType.X, AluOpType.add
                )
                nc.vector.tensor_copy(ds_sb[:, 1:2], acc_ps[:, P : P + 1])

                d_ps = psum_sm.tile([1, C], F32, tag="d")
                s_ps = psum_sm.tile([1, C], F32, tag="s")
                nc.tensor.matmul(
                    d_ps[:], ds_sb[:, 0:1], g4_sb[:], start=True, stop=True
                )
                nc.tensor.matmul(
                    s_ps[:], ds_sb[:, 1:2], g4_sb[:], start=True, stop=True
                )

                cinv_i = cinv_sb[0:1, i : i + 1]
                mean_sb = small.tile([1, C], F32, tag="mean")
                ex2_sb = small.tile([1, C], F32, tag="ex2")
                var_sb = small.tile([1, C], F32, tag="var")
                std_sb = small.tile([1, C], F32, tag="std")
                istd_sb = small.tile([1, C], F32, tag="istd")
                tmp_sb = small.tile([1, C], F32, tag="tmp")
                ab4_sb = small.tile([1, 8 * C], F32, tag="ab4")

                nc.vector.tensor_scalar_mul(mean_sb[:], s_ps[:], cinv_i)
                nc.vector.tensor_scalar_mul(ex2_sb[:], d_ps[:], cinv_i)
                nc.vector.tensor_mul(var_sb[:], mean_sb[:], mean_sb[:])
                nc.vector.tensor_sub(var_sb[:], ex2_sb[:], var_sb[:])
                nc.scalar.activation(
                    std_sb[:],
                    var_sb[:],
                    mybir.ActivationFunctionType.Sqrt,
                    bias=eps_sb[:],
                )
                nc.vector.reciprocal(istd_sb[:], std_sb[:])
                nc.vector.tensor_mul(ab4_sb[:, 0:C], istd_sb[:], w_sb[:])
                nc.vector.tensor_mul(tmp_sb[:], mean_sb[:], ab4_sb[:, 0:C])
                nc.vector.tensor_sub(ab4_sb[:, 4 * C : 5 * C], b_sb[:], tmp_sb[:])
                # replicate A to slots 1..3 and B to slots 5..7
                nc.vector.tensor_copy(ab4_sb[:, C : 2 * C], ab4_sb[:, 0:C])
                nc.vector.tensor_copy(ab4_sb[:, 2 * C : 4 * C], ab4_sb[:, 0 : 2 * C])
                nc.vector.tensor_copy(
                    ab4_sb[:, 5 * C : 6 * C], ab4_sb[:, 4 * C : 5 * C]
                )
                nc.vector.tensor_copy(
                    ab4_sb[:, 6 * C : 8 * C], ab4_sb[:, 4 * C : 6 * C]
                )

                # broadcast [1, 256] -> [128, 256] via K=1 matmul
                ab_ps = psum_sm.tile([P, 8 * C], F32, tag="abps")
                nc.tensor.matmul(
                    ab_ps[:], ones_row[:], ab4_sb[:], start=True, stop=True
                )
                ab_rep = small.tile([P, 8 * C], F32, tag="abrep")
                nc.scalar.copy(ab_rep[:], ab_ps[:])
                return ab_rep

            def pass2(i, ptiles, ab_rep, defer_after=None, defer_from=10**9):
                # normalize in place + store.  Stores with index >=
                # defer_from get an artificial dependency on `defer_after`
                # (the other instance's last load) so the loads get full
                # HBM bandwidth; the deferred stores run in the DMA-idle
                # window that follows.
                a_wide = ab_rep[:, 0 : 4 * C]        # [128, 128]
                b_wide = ab_rep[:, 4 * C : 8 * C]    # [128, 128]
                for p, (pt, r0, rows, rpp, nwt) in enumerate(ptiles):
                    pv = pt[:].rearrange("q (n k) -> q n k", k=P)[:, :nwt, :]
                    a_b = a_wide[:, None, :].broadcast_to([P, nwt, P])
                    b_b = b_wide[:, None, :].broadcast_to([P, nwt, P])
                    nc.vector.tensor_tensor(pv, pv, a_b, AluOpType.mult)
                    nc.vector.tensor_tensor(pv, pv, b_b, AluOpType.add)
                    dst = y_ext[i, r0 : r0 + rows, :].rearrange(
                        "(q n) c -> q (n c)", q=P
                    )
                    st = nc.sync.dma_start(out=dst, in_=pt[:, : rpp * C])
                    if p >= defer_from and defer_after is not None:
                        add_dep_helper(
                            st.ins,
                            defer_after,
                            sync=True,
                            reason="defer store to free HBM bw for loads",
                        )

            # phase order: instance 1's pass-1 is emitted before instance
            # 0's normalize so its loads/casts/matmuls queue ahead and fill
            # the DMA/PE while VectorE runs instance 0's tensor_tensor ops.
            last_load = [None]
            acc0, pt0 = pass1(0, nc.vector.tensor_copy)
            ab0 = stats(0, acc0)
            acc1, pt1 = pass1(1, nc.scalar.copy)
            l1_last = last_load[0]
            # defer_from=5: instance 1 needs 8 panel slots; 3 are free and
            # stores 0..4 release 5 more, so the deferral cannot deadlock.
            pass2(0, pt0, ab0, defer_after=l1_last, defer_from=5)
            ab1 = stats(1, acc1)
            pass2(1, pt1, ab1)

    # Populate .instr bytes for extended-inst InstISA subclasses — raw bass
    # skips this pass and the NEFF compiler fails with "ISA wrong length"
    # on empty .instr.
    mybir.codegen_inst_isa_subclasses(nc)
    _split_waits(nc)
    return nc


def _split_waits(nc, max_waits: int = 1):
    """This container's walrus rejects instructions carrying more than one
    semaphore wait ("Too many sync wait commands").  Hoist extra waits onto
    same-engine InstNoOps inserted just before the instruction.
    """
    for f in nc.m.functions:
        for blk in f.blocks:
            new = []
            for inst in blk.instructions:
                si = inst.sync_info
                if (
                    si is not None
                    and len(si.on_wait) > max_waits
                    and not isinstance(inst, mybir.InstNoOp)
                ):
                    waits = list(si.on_wait)
                    for w in waits[:-max_waits]:
                        nop = mybir.InstNoOp(
                            name=nc.get_next_instruction_name(),
                            engine=inst.engine,
                            sync_info=mybir.SyncInfo(on_wait=[w], on_update=[]),
                            bass_nofuse=True,
                        )
                        new.append(nop)
                    inst.sync_info = mybir.SyncInfo(
                        on_wait=waits[-max_waits:], on_update=list(si.on_update)
                    )
                new.append(inst)
            blk.instructions[:] = new


def _get_program(L: int):
    prog = _PROG_CACHE.get(L)
    if prog is None:
        prog = _build_program(L)
        _PROG_CACHE[L] = prog
    return prog


def _run(feat, batch_ids, weight, bias, trace=False, trace_kwargs=None):
    feat = np.ascontiguousarray(np.asarray(feat, dtype=np.float32))
    batch_ids = np.asarray(batch_ids, dtype=np.int32)
    weight = np.asarray(weight, dtype=np.float32).reshape(1, C)
    bias = np.asarray(bias, dtype=np.float32).reshape(1, C)
    n = feat.shape[0]

    perm = None
    if np.any(np.diff(batch_ids) < 0):  # insurance; spec says sorted
        perm = np.argsort(batch_ids, kind="stable")
        feat = feat[perm]
        batch_ids = batch_ids[perm]

    counts = np.bincount(batch_ids, minlength=N_INSTANCES).astype(np.int64)
    starts = np.zeros(N_INSTANCES + 1, dtype=np.int64)
    np.cumsum(counts, out=starts[1:])

    L = int(max(512, ((counts.max() + 511) // 512) * 512))
    nc = _get_program(L)

    g4 = np.tile(np.eye(C, dtype=np.float32), (4, 1))
    dmask = np.eye(P, dtype=np.float32)

    in_maps = []
    for k in range(NCORES):
        x_pad = np.zeros((IPC, L, C), dtype=np.float32)
        cinv = np.zeros((1, IPC), dtype=np.float32)
        for j in range(IPC):
            inst = k * IPC + j
            s, e = starts[inst], starts[inst + 1]
            x_pad[j, : e - s] = feat[s:e]
            cinv[0, j] = 1.0 / max(float(counts[inst]), 1.0)
        in_maps.append(
            {
                "x": x_pad,
                "wvec": weight,
                "bvec": bias,
                "cinv": cinv,
                "g4": g4,
                "dmask": dmask,
            }
        )

    res = run_bass_kernel_spmd(
        nc,
        in_maps,
        list(range(NCORES)),
        trace=trace,
        **(trace_kwargs or {}),
    )

    out = np.empty((n, C), dtype=np.float32)
    for k in range(NCORES):
        y = res.results[k]["y"]
        for j in range(IPC):
            inst = k * IPC + j
            s, e = starts[inst], starts[inst + 1]
            out[s:e] = y[j, : e - s]

    if perm is not None:
        inv = np.empty_like(perm)
        inv[perm] = np.arange(n)
        out = out[inv]
    return out, res


def kernel(feat, batch_ids, weight, bias):
    out, _ = _run(feat, batch_ids, weight, bias, trace=False)
    return out


# revision 28
# speedup vs baseline: 1.1299x; 1.0205x over previous
"""MinkowskiInstanceNorm on 8 Trainium2 NeuronCores.

Strategy: batch_ids are sorted, so the 16 instances are contiguous row
ranges.  Assign 2 instances per core (no cross-core collectives).  On
host, pad each instance to a uniform length L (zeros) so the SPMD
program is identical across cores.  Per instance on device:
  pass 1: DMA rows into SBUF fp32 panels (each partition holds a
          contiguous row range -> fully contiguous HBM transfers).
          ScalarE casts each panel to a bf16 shadow with a ones column
          per 128-col wide tile; TensorE then computes Gram + sums in a
          single cheap bf16 matmul per wide tile, accumulated in PSUM
          (diag = per-(group,channel) sum of squares, ones col = sums).
          Zero padding contributes nothing.
  stats:  extract diag + sums, fold the 4 row groups per channel with
          two tiny matmuls, compute A = weight/sqrt(var+eps),
          B = bias - mean*A, broadcast to [128, 128] via a K=1 matmul.
  pass 2: y = x*A + B with two VectorE tensor_tensor ops per panel
          (in place on the fp32 panel), then DMA out.  Rows are read
          from HBM once and written once (~64 MB per core).
"""

import numpy as np

import concourse.bass as bass
import concourse.mybir as mybir
import concourse.tile as tile
import concourse.tile_utils as tile_utils
from concourse.alu_op_type import AluOpType
from concourse.bass_utils import run_bass_kernel_spmd
from concourse.tile_rust import add_dep_helper

F32 = mybir.dt.float32
BF16 = mybir.dt.bfloat16
P = 128            # SBUF partitions
C = 32             # channels
IPC = 2            # instances per core
NCORES = 8
N_INSTANCES = 16
EPS = 1e-8
ROWS_PP = 16384    # rows per full panel (128 rows per partition, 2 MiB)

# use the full usable SBUF (the default cap leaves 16 KiB/partition unused)
tile_utils.max_sbuf_usage = 208 * 1024

_PROG_CACHE: dict = {}


def _build_program(L: int):
    """Build the SPMD bass program for instance slot length L (mult of 512)."""
    n_panels = (L + ROWS_PP - 1) // ROWS_PP
    nc = bass.Bass()

    x_ext = nc.declare_dram_parameter("x", [IPC, L, C], F32, isOutput=False)
    w_ext = nc.declare_dram_parameter("wvec", [1, C], F32, isOutput=False)
    b_ext = nc.declare_dram_parameter("bvec", [1, C], F32, isOutput=False)
    g4_ext = nc.declare_dram_parameter("g4s", [P, IPC * C], F32, isOutput=False)
    dm_ext = nc.declare_dram_parameter("dmask", [P, P], F32, isOutput=False)
    y_ext = nc.declare_dram_parameter("y", [IPC, L, C], F32, isOutput=True)

    with tile.TileContext(nc) as tc:
        with (
            tc.tile_pool(name="panels", bufs=11) as panels,
            tc.tile_pool(name="bfp", bufs=1) as bfp,
            tc.tile_pool(name="consts", bufs=1) as consts,
            tc.tile_pool(name="small", bufs=2) as small,
            tc.tile_pool(name="psum_acc", bufs=2, space="PSUM") as psum_acc,
            tc.tile_pool(name="psum_sm", bufs=1, space="PSUM") as psum_sm,
        ):
            w_sb = consts.tile([1, C], F32, tag="w")
            nc.sync.dma_start(out=w_sb[:], in_=w_ext[:])
            b_sb = consts.tile([1, C], F32, tag="b")
            nc.sync.dma_start(out=b_sb[:], in_=b_ext[:])
            g4_sb = consts.tile([P, IPC * C], F32, tag="g4")
            nc.sync.dma_start(out=g4_sb[:], in_=g4_ext[:])
            dm_sb = consts.tile([P, P], F32, tag="dmask")
            nc.sync.dma_start(out=dm_sb[:], in_=dm_ext[:])
            ones_row = consts.tile([1, P], F32, tag="ones_row")
            nc.vector.memset(ones_row[:], 1.0)
            eps_sb = consts.tile([1, 1], F32, tag="eps")
            nc.vector.memset(eps_sb[:], float(EPS))

            # 4 persistent bf16 shadow tiles (double-buffered by WAR deps);
            # ones columns are written once and never overwritten
            CH = 16   # wide tiles per cast chunk (half panel)
            NBF = 4
            bf_tiles = []
            for t in range(NBF):
                bt = consts.tile([P, CH * 130], BF16, tag=f"bf{t}")
                bt3 = bt[:].rearrange("q (n k) -> q n k", k=130)
                nc.vector.memset(bt3[:, :, 128:129], 1.0)
                bf_tiles.append(bt)
            chunk_ctr = [0]

            def pass1(i, cast):
                # load, cast to bf16, accumulate Gram+sums on PE
                acc_ps = psum_acc.tile([P, P + 1], F32, tag="acc")
                ptiles = []
                total_wt = L // 512
                wt_done = 0
                for p in range(n_panels):
                    r0 = p * ROWS_PP
                    rows = min(ROWS_PP, L - r0)
                    rpp = rows // P       # rows per partition
                    nwt = rpp // 4        # wide tiles (128 cols each)
                    pt = panels.tile([P, (ROWS_PP // P) * C], F32, tag="panel")
                    # note: each partition q holds rows [r0+q*rpp, r0+(q+1)*rpp)
                    src3 = x_ext[i, r0 : r0 + rows, :].rearrange(
                        "(q n) c -> q n c", q=P
                    )
                    ptiles.append((pt, r0, rows, rpp, nwt))

                    pt3 = pt[:].rearrange("q (n k) -> q n k", k=P)
                    for c0 in range(0, nwt, CH):
                        cw = min(CH, nwt - c0)
                        # load just this chunk's rows (half panel) so the
                        # cast can start before the rest of the panel lands
                        ld = nc.gpsimd.dma_start(
                            out=pt3[:, c0 : c0 + cw, :],
                            in_=src3[:, c0 * 4 : (c0 + cw) * 4, :].rearrange(
                                "q (n f) c -> q n (f c)", f=4
                            ),
                        )
                        last_load[0] = ld.ins
                        bt = bf_tiles[chunk_ctr[0] % NBF]
                        chunk_ctr[0] += 1
                        bt3 = bt[:].rearrange("q (n k) -> q n k", k=130)
                        cast(bt3[:, :cw, 0:P], pt3[:, c0 : c0 + cw, :])
                        for wt in range(cw):
                            nc.tensor.matmul(
                                acc_ps[:],
                                bt[:, wt * 130 : wt * 130 + P],
                                bt[:, wt * 130 : wt * 130 + P + 1],
                                start=(wt_done == 0),
                                stop=(wt_done == total_wt - 1),
                            )
                            wt_done += 1
                assert wt_done == total_wt
                return acc_ps, ptiles

            def stats(i, acc_ps):
                ds_sb = small.tile([P, 2], F32, tag="ds")
                scratch = psum_sm.tile([P, P], F32, tag="scratch")
                nc.vector.tensor_tensor(
                    scratch[:], acc_ps[:, 0:P], dm_sb[:], AluOpType.mult
                )
                nc.vector.tensor_reduce(
                    ds_sb[:, 0:1], scratch[:], mybir.AxisListType.X, AluOpType.add
                )
                nc.vector.tensor_copy(ds_sb[:, 1:2], acc_ps[:, P : P + 1])

                # group-fold matmuls with 1/n baked into g4s on the host:
                # d_ps = E[x^2], s_ps = mean  (both [1, C])
                g4i = g4_sb[:, i * C : (i + 1) * C]
                d_ps = psum_sm.tile([1, C], F32, tag="d")
                s_ps = psum_sm.tile([1, C], F32, tag="s")
                nc.tensor.matmul(
                    d_ps[:], ds_sb[:, 0:1], g4i, start=True, stop=True
                )
                nc.tensor.matmul(
                    s_ps[:], ds_sb[:, 1:2], g4i, start=True, stop=True
                )

                mean_sb = small.tile([1, C], F32, tag="mean")
                var_sb = small.tile([1, C], F32, tag="var")
                std_sb = small.tile([1, C], F32, tag="std")
                istd_sb = small.tile([1, C], F32, tag="istd")
                tmp_sb = small.tile([1, C], F32, tag="tmp")
                ab4_sb = small.tile([1, 8 * C], F32, tag="ab4")

                nc.vector.tensor_copy(mean_sb[:], s_ps[:])
                nc.vector.tensor_mul(var_sb[:], mean_sb[:], mean_sb[:])
                nc.vector.tensor_sub(var_sb[:], d_ps[:], var_sb[:])
                nc.scalar.activation(
                    std_sb[:],
                    var_sb[:],
                    mybir.ActivationFunctionType.Sqrt,
                    bias=eps_sb[:],
                )
                nc.vector.reciprocal(istd_sb[:], std_sb[:])
                nc.vector.tensor_mul(ab4_sb[:, 0:C], istd_sb[:], w_sb[:])
                nc.vector.tensor_mul(tmp_sb[:], mean_sb[:], ab4_sb[:, 0:C])
                nc.vector.tensor_sub(ab4_sb[:, 4 * C : 5 * C], b_sb[:], tmp_sb[:])
                # replicate A to slots 1..3 and B to slots 5..7
                nc.vector.tensor_copy(ab4_sb[:, C : 2 * C], ab4_sb[:, 0:C])
                nc.vector.tensor_copy(ab4_sb[:, 2 * C : 4 * C], ab4_sb[:, 0 : 2 * C])
                nc.vector.tensor_copy(
                    ab4_sb[:, 5 * C : 6 * C], ab4_sb[:, 4 * C : 5 * C]
                )
                nc.vector.tensor_copy(
                    ab4_sb[:, 6 * C : 8 * C], ab4_sb[:, 4 * C : 6 * C]
                )

                # broadcast [1, 256] -> [128, 256] via K=1 matmul
                ab_ps = psum_sm.tile([P, 8 * C], F32, tag="abps")
                nc.tensor.matmul(
                    ab_ps[:], ones_row[:], ab4_sb[:], start=True, stop=True
                )
                ab_rep = small.tile([P, 8 * C], F32, tag="abrep")
                nc.scalar.copy(ab_rep[:], ab_ps[:])
                return ab_rep

            def pass2(i, ptiles, ab_rep, defer_after=None, defer_from=10**9):
                # normalize in place + store.  Stores with index >=
                # defer_from get an artificial dependency on `defer_after`
                # (the other instance's last load) so the loads get full
                # HBM bandwidth; the deferred stores run in the DMA-idle
                # window that follows.
                a_wide = ab_rep[:, 0 : 4 * C]        # [128, 128]
                b_wide = ab_rep[:, 4 * C : 8 * C]    # [128, 128]
                for p, (pt, r0, rows, rpp, nwt) in enumerate(ptiles):
                    pv = pt[:].rearrange("q (n k) -> q n k", k=P)[:, :nwt, :]
                    a_b = a_wide[:, None, :].broadcast_to([P, nwt, P])
                    b_b = b_wide[:, None, :].broadcast_to([P, nwt, P])
                    nc.vector.tensor_tensor(pv, pv, a_b, AluOpType.mult)
                    nc.vector.tensor_tensor(pv, pv, b_b, AluOpType.add)
                    dst = y_ext[i, r0 : r0 + rows, :].rearrange(
                        "(q n) c -> q (n c)", q=P
                    )
                    st = nc.sync.dma_start(out=dst, in_=pt[:, : rpp * C])
                    if p >= defer_from and defer_after is not None:
                        add_dep_helper(
                            st.ins,
                            defer_after,
                            sync=True,
                            reason="defer store to free HBM bw for loads",
                        )

            # phase order: instance 1's pass-1 is emitted before instance
            # 0's normalize so its loads/casts/matmuls queue ahead and fill
            # the DMA/PE while VectorE runs instance 0's tensor_tensor ops.
            last_load = [None]
            acc0, pt0 = pass1(0, nc.vector.tensor_copy)
            ab0 = stats(0, acc0)
            acc1, pt1 = pass1(1, nc.scalar.copy)
            l1_last = last_load[0]
            # defer_from=5: instance 1 needs 8 panel slots; 3 are free and
            # stores 0..4 release 5 more, so the deferral cannot deadlock.
            pass2(0, pt0, ab0, defer_after=l1_last, defer_from=5)
            ab1 = stats(1, acc1)
            pass2(1, pt1, ab1)

    # Populate .instr bytes for extended-inst InstISA subclasses — raw bass
    # skips this pass and the NEFF compiler fails with "ISA wrong length"
    # on empty .instr.
    mybir.codegen_inst_isa_subclasses(nc)
    _split_waits(nc)
    return nc


def _split_waits(nc, max_waits: int = 1):
    """This container's walrus rejects instructions carrying more than one
    semaphore wait ("Too many sync wait commands").  Hoist extra waits onto
    same-engine InstNoOps inserted just before the instruction.
    """
    for f in nc.m.functions:
        for blk in f.blocks:
            new = []
            for inst in blk.instructions:
                si = inst.sync_info
                if (
                    si is not None
                    and len(si.on_wait) > max_waits
                    and not isinstance(inst, mybir.InstNoOp)
                ):
                    waits = list(si.on_wait)
                    for w in waits[:-max_waits]:
                        nop = mybir.InstNoOp(
                            name=nc.get_next_instruction_name(),
                            engine=inst.engine,
                            sync_info=mybir.SyncInfo(on_wait=[w], on_update=[]),
                            bass_nofuse=True,
                        )
                        new.append(nop)
                    inst.sync_info = mybir.SyncInfo(
                        on_wait=waits[-max_waits:], on_update=list(si.on_update)
                    )
                new.append(inst)
            blk.instructions[:] = new


def _get_program(L: int):
    prog = _PROG_CACHE.get(L)
    if prog is None:
        prog = _build_program(L)
        _PROG_CACHE[L] = prog
    return prog


def _run(feat, batch_ids, weight, bias, trace=False, trace_kwargs=None):
    feat = np.ascontiguousarray(np.asarray(feat, dtype=np.float32))
    batch_ids = np.asarray(batch_ids, dtype=np.int32)
    weight = np.asarray(weight, dtype=np.float32).reshape(1, C)
    bias = np.asarray(bias, dtype=np.float32).reshape(1, C)
    n = feat.shape[0]

    perm = None
    if np.any(np.diff(batch_ids) < 0):  # insurance; spec says sorted
        perm = np.argsort(batch_ids, kind="stable")
        feat = feat[perm]
        batch_ids = batch_ids[perm]

    counts = np.bincount(batch_ids, minlength=N_INSTANCES).astype(np.int64)
    starts = np.zeros(N_INSTANCES + 1, dtype=np.int64)
    np.cumsum(counts, out=starts[1:])

    L = int(max(512, ((counts.max() + 511) // 512) * 512))
    nc = _get_program(L)

    g4 = np.tile(np.eye(C, dtype=np.float32), (4, 1))
    dmask = np.eye(P, dtype=np.float32)

    in_maps = []
    for k in range(NCORES):
        x_pad = np.zeros((IPC, L, C), dtype=np.float32)
        g4s = np.zeros((P, IPC * C), dtype=np.float32)
        for j in range(IPC):
            inst = k * IPC + j
            s, e = starts[inst], starts[inst + 1]
            x_pad[j, : e - s] = feat[s:e]
            g4s[:, j * C : (j + 1) * C] = g4 / max(float(counts[inst]), 1.0)
        in_maps.append(
            {
                "x": x_pad,
                "wvec": weight,
                "bvec": bias,
                "g4s": g4s,
                "dmask": dmask,
            }
        )

    res = run_bass_kernel_spmd(
        nc,
        in_maps,
        list(range(NCORES)),
        trace=trace,
        **(trace_kwargs or {}),
    )

    out = np.empty((n, C), dtype=np.float32)
    for k in range(NCORES):
        y = res.results[k]["y"]
        for j in range(IPC):
            inst = k * IPC + j
            s, e = starts[inst], starts[inst + 1]
            out[s:e] = y[j, : e - s]

    if perm is not None:
        inv = np.empty_like(perm)
        inv[perm] = np.arange(n)
        out = out[inv]
    return out, res


def kernel(feat, batch_ids, weight, bias):
    out, _ = _run(feat, batch_ids, weight, bias, trace=False)
    return out
